# revision 62
# baseline (speedup 1.0000x reference)
"""Trainium2 Bass kernel for multi-head self-attention (B=2, N=2048, C=1024, H=16, d=64).

Sharding: 8 cores = 2 batches x 4 head-groups (4 heads each). Each core computes
QKV for its heads (column-sliced W_qkv), full attention over its heads, and a
row-sliced partial of the output projection. Host sums the 4 partials per batch
and adds b_proj.

Device dataflow (per core, all matmuls bf16 with fp32 PSUM accumulation):
  - x^T is loaded [C, N] so Q^T/K^T come out as [head*d, N] (d on partitions),
    which is exactly the lhsT/rhs layout the scores matmul wants.
  - S^T tile [128 keys, 512 queries] = (K^T chunk)^T-matmul(Q^T chunk), K=64
    contraction; the two heads of a pair sit at partition offsets 0/64.
  - softmax skips the max-subtraction (scores are ~N(0,1); exp is safe in fp32)
    so exp(scale*S) is a single ACT pass straight out of PSUM, cast to bf16.
    The exp stream is the second-busiest engine (~133us); scores for group
    k+1 recycle the PSUM banks the exp of group k reads, so per-head exps are
    emitted as soon as that head's two key tiles are scored.
  - AV runs in O[query, d] orientation: lhsT = P^T slice [128 keys, 128
    queries], rhs = V[128 keys, 65] (65th column ones), so each matmul uses
    all 128 output partitions and costs only 65 output rows - half the PE
    time of the O^T orientation.  Column 64 of the accumulator collects the
    softmax denominator on each query's partition.  The four query sub-tiles
    of a block share a PSUM bank, and a start=True matmul clobbers the whole
    bank, so the bank is memset once and all AV matmuls accumulate with
    start=False.  The two key tiles of a group are emitted in different
    groups (lags 3/5) so region re-writes land clear of their semaphores.
  - normalization is per-partition: one reciprocal [128, 2, 4, 1] per head,
    then one zero-stride-broadcast tensor multiply per head (DVE per-op
    overhead dominates smaller ops) - no broadcast matmuls.
  - O[q, 2*64] flips back to O^T pair layout [128, q] with one PE
    identity-transpose per 128-query tile; the PSUM->SBUF copy doubles as
    the V-bias add (b_v commutes through the softmax average).
  - projection: both 512-column halves of a query tile in one deferred task
    ending in a single bf16 store (HWDGE descriptor-gen serializes, so fewer
    bigger stores win); the host sums partials in f32.  The LAST query
    chunk's two contraction halves are separate partial stores: its pair-0
    half runs a whole sweep early, halving the tail's PE work.
  - scheduling: producer chains drip-feed with hand-tuned pacing (front-
    loaded into each block's early groups, spread so pair-1 blocks aren't
    starved), prev-block normalization splits into a DVE part at group 0 and
    a PE part at group 2, the first input chunks are split fine across three
    DMA issue queues, and the PE is HAM-warmed during the initial DMAs.
"""

import sys

sys.path.insert(0, "/opt/trn_rl_repo")

import numpy as np
import ml_dtypes

import concourse.bass as bass
import concourse.tile as tile
from concourse import bacc, masks, mybir
from concourse.bass_utils import run_bass_kernel_spmd

BF16 = ml_dtypes.bfloat16
F32 = mybir.dt.float32
BF = mybir.dt.bfloat16
AF = mybir.ActivationFunctionType

B, NT, C, H, D = 2, 2048, 1024, 16, 64
NCORES = 8
HPC = 4  # heads per core
DQ = HPC * D  # 256 c_out per q/k/v slice
VW = HPC * (D + 1)  # 260: V with a ones column per head
SCALE = D ** -0.5


def build_program(nt=NT):
    """Build the SPMD Bass program. nt parametrized so a small version can be
    simulated quickly in CoreSim."""
    n_tc = nt // 512  # 512-token chunks
    n_kt = nt // 128  # 128-key tiles
    n_ktg = nt // 256  # groups of 2 key tiles (one exp per head per 1024 cols)

    nc = bacc.Bacc("TRN2", target_bir_lowering=False, debug=False,
                   num_devices=NCORES)

    xT = nc.dram_tensor("xT", [C, nt], BF, kind="ExternalInput").ap()
    wq = nc.dram_tensor("wq", [C, DQ], BF, kind="ExternalInput").ap()
    wk = nc.dram_tensor("wk", [C, DQ], BF, kind="ExternalInput").ap()
    wv = nc.dram_tensor("wv", [C, DQ], BF, kind="ExternalInput").ap()
    wp = nc.dram_tensor("wp", [DQ, C], BF, kind="ExternalInput").ap()
    bqk = nc.dram_tensor("bqk", [128, 4], F32, kind="ExternalInput").ap()
    bvp = nc.dram_tensor("bvp", [128, 2], F32, kind="ExternalInput").ap()
    # bf16 output, stored as the two 128-channel contraction halves of the
    # projection; the host sums them (and the core partials) in f32.  Adds
    # ~0.4% rounding per partial, well inside the error budget.
    out = nc.dram_tensor("out_p", [2, nt, C], BF, kind="ExternalOutput").ap()

    with tile.TileContext(nc) as tc:
        with (
            tc.tile_pool(name="persist", bufs=1) as persist,
            tc.tile_pool(name="pt_pool", bufs=20) as pt_pool,
            tc.tile_pool(name="stage", bufs=6) as stage,
            tc.tile_pool(name="osb_pool", bufs=2) as osb_pool,
            tc.tile_pool(name="rc_pool", bufs=2) as rc_pool,
            tc.tile_pool(name="ps_qkv", bufs=2, space="PSUM") as ps_qkv,
            tc.tile_pool(name="ps_s", bufs=2, space="PSUM") as ps_s,
            tc.tile_pool(name="ps_o", bufs=1, space="PSUM") as ps_o,
        ):
            # ---------------- persistent SBUF state ----------------
            # load order matters: wk + xT feed the first K^T chains; wv/wp
            # are only needed once attention is underway.
            # x^T lives in TWO tiles split at token 512: the tile framework
            # approximates write regions as bounding byte-ranges, so a late
            # chunk write like [po 0:4, 512:1024] would falsely alias the
            # first block's [po 1, 0:512] reads and stall the q00 chain on
            # DMAs it doesn't need.  A separate first-block tile kills that.
            xTa_sb = persist.tile([128, 8, min(nt, 512)], BF)
            if nt > 512:
                xTb_sb = persist.tile([128, 8, nt - 512], BF, name="xTb_sb")
            else:
                xTb_sb = None

            def xT_ap(ci, lo, hi):
                # [128, hi-lo] view of x^T chunk ci, tokens [lo, hi)
                if hi <= 512:
                    return xTa_sb[:, ci, lo:hi]
                assert lo >= 512
                return xTb_sb[:, ci, lo - 512:hi - 512]
            wq_sb = persist.tile([128, 8, DQ], BF)
            wk_sb = persist.tile([128, 8, DQ], BF)
            wv_sb = persist.tile([128, 8, DQ], BF)
            bqk_sb = persist.tile([128, 4], F32)
            bvp_sb = persist.tile([128, 2], F32)
            wp_sb = persist.tile([128, 2, C], BF)
            # Few big DMA instructions (the ~1.3us sequencer issue cost per
            # DMA dominates; transfers run on 16 parallel DMA engines).
            # x^T rides the SP hardware queue in 512-token chunks so the first
            # K/Q chains start early; weights ride the idle Pool (SWDGE) queue.
            xT3 = xT.rearrange("(po pi) n -> pi po n", pi=128)
            wk3 = wk.rearrange("(po pi) c -> pi po c", pi=128)
            wq3 = wq.rearrange("(po pi) c -> pi po c", pi=128)
            wv3 = wv.rearrange("(po pi) c -> pi po c", pi=128)
            wp3 = wp.rearrange("(po pi) c -> pi po c", pi=128)
            def _xt(t):
                if t == 0:
                    return
                sl = slice(t * 512, (t + 1) * 512)
                slb = slice(t * 512 - 512, (t + 1) * 512 - 512)
                nc.sync.dma_start(xTb_sb[:, 0:4, slb], xT3[:, 0:4, sl])
                nc.sync.dma_start(xTb_sb[:, 4:8, slb], xT3[:, 4:8, sl])

            ones_sb = persist.tile([1, 512], BF)
            nc.vector.memset(ones_sb[:], 1.0)
            # warm the PE clock (HAM) immediately: a few tiny matmuls up
            # front get the clock ramp going while the first DMAs land
            warm_ps = ps_qkv.tile([128, 512], F32, tag="qkv", name="warm_ps")
            for i in range(8):
                nc.tensor.matmul(warm_ps[:, 0:256], ones_sb[:, 0:128],
                                 ones_sb[:, 0:256], start=(i == 0),
                                 stop=(i == 7), skip_group_check=True)
            # NOTE: no SBUF read-back of warm_ps: a DVE copy here would wait
            # for the last bridge warm inside the k00 chain and head-of-line
            # block the DVE queue ahead of the k00/q00 bias adds, delaying
            # the first exp by ~2.5us.

            # first weight/token chunks spread across FOUR issue queues (SP,
            # Pool/SWDGE, DVE, ACT) so the first K-chain's inputs all land by
            # ~2.5us instead of trickling off one serialized SP queue
            # (565ns/issue).  Tokens 0-511 must be complete in ALL 8
            # contraction chunks before the first chains' late matmuls.
            # The SWDGE (gpsimd) path transfers on its own track, adding
            # bandwidth in parallel with the HWDGE pool: it carries part of
            # the critical first token block plus wv and the late token
            # chunks, so the HWDGE pool can focus on weights + early tokens.
            # bqk rides the SWDGE queue FIRST (it gates the k00/q00 bias
            # adds); the first token block's xT streams on SWDGE in parallel
            # with the weights on HWDGE, so wq[4:8] (the last input of the
            # q00 chain) lands by ~5us instead of ~6.5us.
            nc.sync.dma_start(wk_sb[:, 0:4, :], wk3[:, 0:4, :])
            nc.gpsimd.dma_start(xTa_sb[:, 0:2, :], xT3[:, 0:2, 0:512])
            nc.scalar.dma_start(xTa_sb[:, 2:4, :], xT3[:, 2:4, 0:512])
            nc.sync.dma_start(wk_sb[:, 4:8, :], wk3[:, 4:8, :])
            nc.scalar.dma_start(xTa_sb[:, 4:6, :], xT3[:, 4:6, 0:512])
            nc.gpsimd.dma_start(bqk_sb[:], bqk)
            nc.sync.dma_start(xTa_sb[:, 6:8, :], xT3[:, 6:8, 0:512])
            nc.sync.dma_start(wq_sb[:, 0:4, :], wq3[:, 0:4, :])
            nc.sync.dma_start(wq_sb[:, 4:8, :], wq3[:, 4:8, :])
            # wv AFTER wq on the same queue: an early SWDGE-issued wv slots
            # its 512KB transfer into the shared DMA track ahead of wq and
            # delays the q00 chain (and the first exp) by ~2us.
            nc.sync.dma_start(wv_sb[:], wv3)
            for t in range(1, n_tc):
                _xt(t)
                if t == 1:
                    nc.sync.dma_start(bvp_sb[:], bvp)
                elif t == 2:
                    nc.sync.dma_start(wp_sb[:], wp3)
            if n_tc < 3:
                nc.sync.dma_start(bvp_sb[:], bvp)
                nc.sync.dma_start(wp_sb[:], wp3)
            ident_sb = persist.tile([128, 128], BF)
            masks.make_identity(nc, ident_sb[:])

            qT_sb = [persist.tile([128, nt], BF, tag=f"qT{p}", name=f"qT{p}")
                     for p in range(2)]
            kT_sb = [persist.tile([128, nt], BF, tag=f"kT{p}", name=f"kT{p}")
                     for p in range(2)]
            oT_sb = [persist.tile([128, n_tc, 4, 128], BF, tag=f"oT{p}",
                                  name=f"oT{p}")
                     for p in range(2)]
            # V layout [keys, kt, head, 65]: col 64 of each head is a ones
            # column (preset once) so AV accumulates softmax denominators.
            v_sb = persist.tile([128, n_kt, HPC, D + 1], BF)
            nc.vector.memset(v_sb[:, :, :, 64:65], 1.0)

            # ---------------- QKV chain emitters ----------------
            def qk_chain(w_sb, bcol, dst, p, t, warm_after=None):
                # warm_after={ci: n} interleaves n dummy warm matmuls after
                # chunk ci: during the preamble the chain stalls on DMA
                # arrivals, and any PE idle >100ns drops the clock back to a
                # low p-state; the dummies keep the busy-streak alive so the
                # real matmuls stay at full speed.
                ps = ps_qkv.tile([128, 512], F32, tag="qkv")
                for ci in range(8):
                    nc.tensor.matmul(
                        ps[:, :],
                        w_sb[:, ci, p * 128:(p + 1) * 128],
                        xT_ap(ci, t * 512, (t + 1) * 512),
                        start=(ci == 0), stop=(ci == 7))
                    if warm_after and ci in warm_after:
                        for _ in range(warm_after[ci]):
                            nc.tensor.matmul(
                                warm_ps[:, 0:256], ones_sb[:, 0:128],
                                ones_sb[:, 0:256], start=True, stop=True,
                                skip_group_check=True)
                nc.vector.tensor_scalar_add(dst[:, t * 512:(t + 1) * 512],
                                            ps[:, :], bqk_sb[:, bcol:bcol + 1])

            def v_chain(tt):
                ps = ps_qkv.tile([128, 8, 64], F32, tag="qkv")
                for ci in range(8):
                    nc.tensor.matmul(
                        ps[:, 0:4, :],
                        xT_ap(ci, tt * 128, (tt + 1) * 128),
                        wv_sb[:, ci, :],
                        start=(ci == 0), stop=(ci == 7))
                # v bias is added after normalization (it commutes through
                # the softmax average), via the oT copy's per-partition add
                nc.vector.tensor_copy(v_sb[:, tt, :, 0:64], ps[:, 0:4, :])

            def qk_half(w_sb, bcol, dst, p, t, half):
                # 256-token half chain: a finer-grained (853ns) PE insert
                # for pacing the overloaded early blocks
                ps = ps_qkv.tile([128, 256], F32, tag="qkv", name="ps_h")
                lo = t * 512 + half * 256
                for ci in range(8):
                    nc.tensor.matmul(
                        ps[:, :],
                        w_sb[:, ci, p * 128:(p + 1) * 128],
                        xT_ap(ci, lo, lo + 256),
                        start=(ci == 0), stop=(ci == 7))
                nc.vector.tensor_scalar_add(dst[:, lo:lo + 256], ps[:, :],
                                            bqk_sb[:, bcol:bcol + 1])

            # bqk_sb columns: 0,1 = q bias pair 0/1; 2,3 = k bias pair 0/1
            def k_chain(p, t, warm_after=None):
                qk_chain(wk_sb, 2 + p, kT_sb[p], p, t, warm_after)

            def q_chain(p, t, warm_after=None):
                qk_chain(wq_sb, 0 + p, qT_sb[p], p, t, warm_after)

            def K2(p, t, half):
                return lambda: qk_half(wk_sb, 2 + p, kT_sb[p], p, t, half)

            def Q2(p, t, half):
                return lambda: qk_half(wq_sb, 0 + p, qT_sb[p], p, t, half)

            # Preamble: just enough for the first attention group to start.
            k_chain(0, 0, warm_after={3: 3, 5: 4})
            q_chain(0, 0)

            # Static insert schedule: each producer chain is emitted 2+
            # groups before its first consumer needs it.  K0 chains feed
            # block 0's own key sweep (forced early); V chains feed the
            # lagged AV pops (spread over blocks 0-1); K1/Q1 chains land
            # just-in-time in blocks 2-5 where the PE has slack.
            def V(tt):
                return lambda: v_chain(tt)

            def K(p, t):
                return lambda: k_chain(p, t)

            def Q(p, t):
                return lambda: q_chain(p, t)

            # ---------------- attention + projection ----------------
            out4 = out.rearrange("pp q (a c) -> pp q a c", a=2)

            def make_proj(qt, pp=None, copy=("dve", "dve"), tail_pool=False,
                          store="sync"):
                # Both column halves of a query tile in one task so the
                # store is a single DMA (HWDGE descriptor-gen serializes).
                # pp=None: both contraction halves accumulated (stored in
                # partial slot 0).  For the LAST query chunk only, the two
                # halves are separate partial stores summed on the host:
                # its pp=0 half only needs pair-0's oT, ready a whole sweep
                # early, so the tail's PE work halves.  `copy` picks the
                # PSUM->SBUF copy engine per half ("act"/"dve"/"pool") and
                # `store` the DMA issue queue, so the end-of-kernel tasks
                # can spread across the otherwise-idle engines.
                def _copy(eng, dst, src):
                    if eng == "act":
                        nc.scalar.copy(dst, src)
                    elif eng == "pool":
                        nc.gpsimd.tensor_copy(dst, src)
                    else:
                        nc.vector.tensor_copy(dst, src)

                def proj():
                    ost = stage.tile([128, 2, 512], BF, tag="ost", name="ost")
                    for nh in range(2):
                        pool = ps_s if (tail_pool and nh == 1) else ps_qkv
                        pps = pool.tile(
                            [128, 512], F32,
                            tag="s" if pool is ps_s else "qkv", name="pps")
                        for ppi in ([0, 1] if pp is None else [pp]):
                            nc.tensor.matmul(
                                pps[:, :],
                                oT_sb[ppi][:, qt // 4, qt % 4, :],
                                wp_sb[:, ppi, nh * 512:(nh + 1) * 512],
                                start=(pp is not None or ppi == 0),
                                stop=(pp is not None or ppi == 1))
                        _copy(copy[nh], ost[:, nh, :], pps[:, :])
                    eng = {"swdge": nc.gpsimd, "scalar": nc.scalar,
                           "sync": nc.sync}[store]
                    eng.dma_start(
                        out4[pp or 0, qt * 128:(qt + 1) * 128, :, :],
                        ost[:, :, :])
                return proj

            def make_norm_dve(o_ps, rc_sb, O_sb):
                # normalization multiplies; emitted at the START of the next
                # block so the DVE has them done well before the transposes.
                # One zero-stride-broadcast multiply per head instead of 8
                # per-subtile ops: the DVE per-op overhead dominates there.
                def norm_dve():
                    for hh in range(2):
                        nc.vector.tensor_mul(
                            O_sb[:, :, hh, :],
                            o_ps[hh][:, :, 0:64],
                            rc_sb[:, hh, :, :].broadcast_to([128, 4, 64]))
                return norm_dve

            def make_norm_pe(O_sb, p, qc, last_block):
                # transpose O back to O^T pair layout + oT copy (with the
                # deferred v-bias add); emitted 2 groups into the next block.
                def norm_pe():
                    tr = ps_qkv.tile([128, 4, 128], BF, tag="qkv",
                                     name="tr_ps")
                    for qi in range(4):
                        nc.tensor.transpose(tr[:, qi, :], O_sb[:, qi, :, :],
                                            ident_sb[:, :])
                    nc.vector.tensor_scalar_add(oT_sb[p][:, qc, :, :],
                                                tr[:, :, :],
                                                bvp_sb[:, p:p + 1])
                    # queue projections now that oT[p] is written: fused
                    # tasks for pair-1 chunks; for the last query chunk its
                    # pair-0 partial is queued as soon as the pair-0 sweep
                    # finishes, so the tail only runs its pair-1 half
                    if p == 1 and not last_block:
                        for qt4 in range(4):
                            deferred.append(make_proj(qc * 4 + qt4))
                    elif p == 0 and qc == n_tc - 1:
                        for qt4 in range(4):
                            deferred.append(make_proj(qc * 4 + qt4, 0))
                return norm_pe

            deferred = []
            blocks = [(qc, 0) for qc in range(n_tc)] + \
                     [(qc, 1) for qc in range(n_tc)]
            nb = len(blocks)
            carry = (n_tc == 4 and n_ktg == 8)
            if carry:
                # (block, group) -> producer-chain thunks, each placed 2+
                # groups before its first consumer.
                sched = {
                    (0, 0): [V(0), K2(0, 1, 0)], (0, 1): [K2(0, 1, 1), V(1)],
                    (0, 2): [V(2), K2(0, 2, 0)], (0, 3): [K2(0, 2, 1), V(3)],
                    (0, 4): [V(4), K2(0, 3, 0)], (0, 5): [K2(0, 3, 1), V(5)],
                    (0, 6): [Q(0, 1), V(6)], (0, 7): [V(7), V(8)],
                    (1, 0): [V(9), V(10)], (1, 1): [V(11), V(12)],
                    (1, 2): [V(13), V(14)], (1, 3): [V(15)],
                    (1, 5): [Q(0, 2)],
                    (2, 4): [Q(0, 3)], (2, 5): [K(1, 0)], (2, 6): [Q(1, 0)],
                    (3, 2): [K(1, 1)], (3, 4): [K(1, 2)], (3, 6): [Q(1, 1)],
                    (4, 2): [K(1, 3)], (4, 5): [Q(1, 2)],
                    (5, 5): [Q(1, 3)],
                }

                # AV pops per (block, group).  Steady state: the 9 carried
                # halves of block bi-1 drain at g0-g4; own pops resume after
                # the g5 memset; block bi hands 9 halves to block bi+1.  The
                # last two blocks pop faster so the final block's norm can
                # run at its g3/g4 (freeing its projections early) and the
                # loop ends with every AV half already emitted - only the
                # exp(g7)-gated kt14/kt15 execute after the last exp.
                def pops_for(bi, g):
                    if bi == 0:
                        return (1 if g >= 4 else 0, 1 if g >= 5 else 0)
                    if bi == nb - 1 and g >= 5:
                        return (2, 2)
                    return {0: (1, 1), 1: (1, 1), 2: (1, 1), 3: (1, 1),
                            4: (0, 1), 5: (1, 1), 6: (1, 1), 7: (2, 1)}[g]
            else:
                # small-sim fallback: dense schedule, in-block drain
                sched = {}
                fill = ([V(tt) for tt in range(2 * n_ktg)]
                        + [K(0, t) for t in range(1, n_tc)]
                        + [Q(0, t) for t in range(1, n_tc)]
                        + [K(1, t) for t in range(n_tc)]
                        + [Q(1, t) for t in range(n_tc)])
                gi = 0
                while fill:
                    b, g = divmod(gi % (nb * n_ktg), n_ktg)
                    sched.setdefault((b, g), []).append(fill.pop(0))
                    gi += 1
                j0l = min(2, n_ktg - 1)
                j1l = min(3, n_ktg - 1)

                def pops_for(bi, g):
                    return (1 if g >= j0l else 0, 1 if g >= j1l else 0)

            def make_av_half(ktg, pt, j, o_ps, p):
                # one key tile's 8 AV matmuls (both heads, 4 query sub-tiles)
                def av():
                    kt = ktg * 2 + j
                    for hh in range(2):
                        h = 2 * p + hh
                        for qi in range(4):
                            nc.tensor.matmul(
                                o_ps[hh][:, qi, :],
                                pt[hh][:, j * 512 + qi * 128:
                                       j * 512 + (qi + 1) * 128],
                                v_sb[:, kt, h, :],
                                start=False,
                                stop=(kt == 2 * n_ktg - 1),
                                skip_group_check=True)
                return av

            j0q, j1q = [], []
            pending = None  # previous block's (o_ps, qc, p) awaiting norm
            pend_pe = None
            for bi, (qc, p) in enumerate(blocks):
                o_ps = [ps_o.tile([128, 4, 65], F32, tag=f"o{_h}",
                                  name=f"o_ps{_h}")
                        for _h in range(2)]
                # AV accumulation groups for the 4 query sub-tiles share a
                # PSUM bank; a start=True matmul resets the whole bank, so
                # the bank is memset once (g5, after the prev block's norm
                # reads) and all AV matmuls accumulate with start=False.
                def o_memset(o_ps=o_ps):
                    for _h in range(2):
                        nc.vector.memset(o_ps[_h][:, :, :], 0.0)
                last_block = (bi == nb - 1)
                if bi == 0 or not carry:
                    o_memset()
                for ktg in range(n_ktg):
                    s_ps = [ps_s.tile([128, 1024], F32, tag="s",
                                      name=f"s_ps{_h}")
                            for _h in range(2)]
                    pt = [pt_pool.tile([128, 1024], BF, tag="pt",
                                       name=f"pt{_h}")
                          for _h in range(2)]
                    # hh-major score order: each head's exp is emitted as
                    # soon as that head's two key tiles are scored
                    for hh in range(2):
                        for j in range(2):
                            kt = ktg * 2 + j
                            nc.tensor.matmul(
                                s_ps[hh][:, j * 512:(j + 1) * 512],
                                kT_sb[p][hh * 64:(hh + 1) * 64,
                                         kt * 128:(kt + 1) * 128],
                                qT_sb[p][hh * 64:(hh + 1) * 64,
                                         qc * 512:(qc + 1) * 512],
                                start=True, stop=True)
                        nc.scalar.activation(pt[hh][:, :], s_ps[hh][:, :],
                                             AF.Exp, scale=SCALE)
                    # prev block's normalization: its last carried AV was
                    # emitted at g4 (g2 for the last block), so the
                    # denominators are complete; the memset must follow the
                    # norm's o_ps reads (in-order DVE) and precede this
                    # block's own AV pops.
                    norm_g = 5
                    if carry and pending is not None and ktg == norm_g:
                        po, pqc, pp_ = pending
                        rc_sb = rc_pool.tile([128, 2, 4, 1], F32, tag="rc",
                                             name="rc_sb")
                        O_sb = osb_pool.tile([128, 4, 2, 64], BF, tag="osb",
                                             name="O_sb")
                        for hh in range(2):
                            nc.vector.reciprocal(rc_sb[:, hh, :, :],
                                                 po[hh][:, :, 64:65])
                        make_norm_dve(po, rc_sb, O_sb)()
                        o_memset()
                        pend_pe = make_norm_pe(O_sb, pp_, pqc,
                                               last_block=False)
                        pending = None
                    if carry and pend_pe is not None and ktg == norm_g + 1:
                        pend_pe()
                        pend_pe = None
                    j0q.append(make_av_half(ktg, pt, 0, o_ps, p))
                    j1q.append(make_av_half(ktg, pt, 1, o_ps, p))
                    n0, n1 = pops_for(bi, ktg)
                    for _ in range(n1):
                        if j1q:
                            j1q.pop(0)()
                    for _ in range(n0):
                        if j0q:
                            j0q.pop(0)()
                    ins = sched.get((bi, ktg), [])
                    for th in ins:
                        th()
                    # projection tasks fill otherwise-empty groups, but stay
                    # out of the last block's final groups (they'd delay the
                    # last exps, which gate the whole tail)
                    if not ins and deferred and not (last_block and ktg >= 5):
                        deferred.pop(0)()
                if not carry:
                    while j0q or j1q:
                        if j1q:
                            j1q.pop(0)()
                        if j0q:
                            j0q.pop(0)()
                    if pending is not None:
                        po, pqc, pp_ = pending
                        rc_sb = rc_pool.tile([128, 2, 4, 1], F32, tag="rc",
                                             name="rc_sb")
                        O_sb = osb_pool.tile([128, 4, 2, 64], BF, tag="osb",
                                             name="O_sb")
                        for hh in range(2):
                            nc.vector.reciprocal(rc_sb[:, hh, :, :],
                                                 po[hh][:, :, 64:65])
                        make_norm_dve(po, rc_sb, O_sb)()
                        make_norm_pe(O_sb, pp_, pqc, last_block=False)()
                        pending = None
                pending = (o_ps, qc, p)
            # drain the last block's carried AV halves (4 in the carry
            # schedule); kt14 (j0) then kt15 (j1) land last so the stop
            # flag closes the accumulation group.
            while j0q or j1q:
                if j0q:
                    j0q.pop(0)()
                if j1q:
                    j1q.pop(0)()
            lb_o_ps, lb_qc, _lb_p = pending
            lb_rc = rc_pool.tile([128, 2, 4, 1], F32, tag="rc", name="lb_rc")
            lb_O = osb_pool.tile([128, 4, 2, 64], BF, tag="osb", name="lb_O")
            for hh in range(2):
                nc.vector.reciprocal(lb_rc[:, hh, :, :],
                                     lb_o_ps[hh][:, :, 64:65])
            # tail: last block's normalization; scales prefetched onto the
            # DVE while the PE chews leftover deferred projections, then a
            # hand-interleaved transpose/copy/projection pipeline.
            if deferred:
                deferred.pop(0)()
            # qi0's scales first so the first transpose (and the whole
            # per-qi tail pipeline behind it) starts half an op earlier
            for hh in range(2):
                nc.vector.tensor_scalar_mul(
                    lb_O[:, 0, hh, :], lb_o_ps[hh][:, 0, 0:64],
                    lb_rc[:, hh, 0, :])
            for hh in range(2):
                nc.vector.tensor_mul(
                    lb_O[:, 1:4, hh, :],
                    lb_o_ps[hh][:, 1:4, 0:64],
                    lb_rc[:, hh, 1:4, :].broadcast_to([128, 3, 64]))
            while deferred:
                deferred.pop(0)()
            tr_t = ps_qkv.tile([128, 4, 128], BF, tag="qkv", name="tr_ps_t")

            def tail_tr(qi):
                nc.tensor.matmul(tr_t[:, qi, :], lb_O[:, qi, :, :],
                                 ident_sb[:, :], is_transpose=True,
                                 skip_group_check=True)
                nc.vector.tensor_scalar_add(oT_sb[1][:, lb_qc, qi, :],
                                            tr_t[:, qi, :],
                                            bvp_sb[:, 1:2])

            tail_tr(0)
            tail_tr(1)
            for qi in range(4):
                qt = lb_qc * 4 + qi
                if qi + 2 < 4:
                    tail_tr(qi + 2)
                # the last two stores go out on different queues so the
                # end-of-program drain waits on them in parallel
                make_proj(qt, 1, copy=("act", "dve"), tail_pool=True,
                          store="scalar" if qi >= 2 else "sync")()

    nc.finalize()
    return nc


def make_core_inputs(x, W_qkv, b_qkv, W_proj, nt=NT):
    """Host-side shard prep: returns in_maps list for the 8 cores."""
    in_maps = []
    for core in range(NCORES):
        b, g = divmod(core, NCORES // B)
        lo, hi = g * DQ, (g + 1) * DQ
        xTb = np.ascontiguousarray(x[b].T).astype(BF16)
        wq_c = np.ascontiguousarray(W_qkv[:, lo:hi]).astype(BF16)
        wk_c = np.ascontiguousarray(W_qkv[:, C + lo:C + hi]).astype(BF16)
        wv_c = np.ascontiguousarray(W_qkv[:, 2 * C + lo:2 * C + hi]).astype(BF16)
        bvp_c = np.stack([
            b_qkv[2 * C + lo:2 * C + lo + 128],
            b_qkv[2 * C + lo + 128:2 * C + hi],
        ], axis=1).astype(np.float32)
        wp_c = np.ascontiguousarray(W_proj[lo:hi, :]).astype(BF16)
        bqk_c = np.stack([
            b_qkv[lo:lo + 128], b_qkv[lo + 128:hi],
            b_qkv[C + lo:C + lo + 128], b_qkv[C + lo + 128:C + hi],
        ], axis=1).astype(np.float32)
        in_maps.append({
            "xT": xTb[:, :nt].copy(), "wq": wq_c, "wk": wk_c, "wv": wv_c,
            "wp": wp_c, "bqk": bqk_c, "bvp": bvp_c,
        })
    return in_maps


_prog_cache = {}


def _get_program(nt=NT):
    if nt not in _prog_cache:
        _prog_cache[nt] = build_program(nt)
    return _prog_cache[nt]


def kernel(x, W_qkv, b_qkv, W_proj, b_proj, _run_kwargs=None):
    x = np.asarray(x, dtype=np.float32)
    W_qkv = np.asarray(W_qkv, dtype=np.float32)
    b_qkv = np.asarray(b_qkv, dtype=np.float32)
    W_proj = np.asarray(W_proj, dtype=np.float32)
    b_proj = np.asarray(b_proj, dtype=np.float32)

    nc = _get_program()
    in_maps = make_core_inputs(x, W_qkv, b_qkv, W_proj)
    for attempt in range(3):
        res = run_bass_kernel_spmd(nc, in_maps, core_ids=list(range(NCORES)),
                                   **(_run_kwargs or {}))
        out = np.zeros((B, NT, C), dtype=np.float32)
        for core in range(NCORES):
            b = core // (NCORES // B)
            part = np.asarray(res.results[core]["out_p"], dtype=np.float32)
            out[b] += part[0]
            # only the last query chunk is stored as two separate halves
            out[b, (NT // 4) * 3:] += part[1][(NT // 4) * 3:]
        if np.isfinite(out).all():
            break
        # transient device flake (observed rarely under axon): retry
    out += b_proj[None, None, :]
    if _run_kwargs:
        kernel.last_results = res
    return out



# revision 63
# speedup vs baseline: 1.0009x; 1.0009x over previous
"""Trainium2 Bass kernel for multi-head self-attention (B=2, N=2048, C=1024, H=16, d=64).

Sharding: 8 cores = 2 batches x 4 head-groups (4 heads each). Each core computes
QKV for its heads (column-sliced W_qkv), full attention over its heads, and a
row-sliced partial of the output projection. Host sums the 4 partials per batch
and adds b_proj.

Device dataflow (per core, all matmuls bf16 with fp32 PSUM accumulation):
  - x^T is loaded [C, N] so Q^T/K^T come out as [head*d, N] (d on partitions),
    which is exactly the lhsT/rhs layout the scores matmul wants.
  - S^T tile [128 keys, 512 queries] = (K^T chunk)^T-matmul(Q^T chunk), K=64
    contraction; the two heads of a pair sit at partition offsets 0/64.
  - softmax skips the max-subtraction (scores are ~N(0,1); exp is safe in fp32)
    so exp(scale*S) is a single ACT pass straight out of PSUM, cast to bf16.
    The exp stream is the second-busiest engine (~133us); scores for group
    k+1 recycle the PSUM banks the exp of group k reads, so per-head exps are
    emitted as soon as that head's two key tiles are scored.
  - AV runs in O[query, d] orientation: lhsT = P^T slice [128 keys, 128
    queries], rhs = V[128 keys, 65] (65th column ones), so each matmul uses
    all 128 output partitions and costs only 65 output rows - half the PE
    time of the O^T orientation.  Column 64 of the accumulator collects the
    softmax denominator on each query's partition.  The four query sub-tiles
    of a block share a PSUM bank, and a start=True matmul clobbers the whole
    bank, so the bank is memset once and all AV matmuls accumulate with
    start=False.  The two key tiles of a group are emitted in different
    groups (lags 3/5) so region re-writes land clear of their semaphores.
  - normalization is per-partition: one reciprocal [128, 2, 4, 1] per head,
    then one zero-stride-broadcast tensor multiply per head (DVE per-op
    overhead dominates smaller ops) - no broadcast matmuls.
  - O[q, 2*64] flips back to O^T pair layout [128, q] with one PE
    identity-transpose per 128-query tile; the PSUM->SBUF copy doubles as
    the V-bias add (b_v commutes through the softmax average).
  - projection: both 512-column halves of a query tile in one deferred task
    ending in a single bf16 store (HWDGE descriptor-gen serializes, so fewer
    bigger stores win); the host sums partials in f32.  The LAST query
    chunk's two contraction halves are separate partial stores: its pair-0
    half runs a whole sweep early, halving the tail's PE work.
  - scheduling: producer chains drip-feed with hand-tuned pacing (front-
    loaded into each block's early groups, spread so pair-1 blocks aren't
    starved), prev-block normalization splits into a DVE part at group 0 and
    a PE part at group 2, the first input chunks are split fine across three
    DMA issue queues, and the PE is HAM-warmed during the initial DMAs.
"""

import sys

sys.path.insert(0, "/opt/trn_rl_repo")

import numpy as np
import ml_dtypes

import concourse.bass as bass
import concourse.tile as tile
from concourse import bacc, masks, mybir
from concourse.bass_utils import run_bass_kernel_spmd

BF16 = ml_dtypes.bfloat16
F32 = mybir.dt.float32
BF = mybir.dt.bfloat16
AF = mybir.ActivationFunctionType

B, NT, C, H, D = 2, 2048, 1024, 16, 64
NCORES = 8
HPC = 4  # heads per core
DQ = HPC * D  # 256 c_out per q/k/v slice
VW = HPC * (D + 1)  # 260: V with a ones column per head
SCALE = D ** -0.5


def build_program(nt=NT):
    """Build the SPMD Bass program. nt parametrized so a small version can be
    simulated quickly in CoreSim."""
    n_tc = nt // 512  # 512-token chunks
    n_kt = nt // 128  # 128-key tiles
    n_ktg = nt // 256  # groups of 2 key tiles (one exp per head per 1024 cols)

    nc = bacc.Bacc("TRN2", target_bir_lowering=False, debug=False,
                   num_devices=NCORES)

    xT = nc.dram_tensor("xT", [C, nt], BF, kind="ExternalInput").ap()
    wq = nc.dram_tensor("wq", [C, DQ], BF, kind="ExternalInput").ap()
    wk = nc.dram_tensor("wk", [C, DQ], BF, kind="ExternalInput").ap()
    wv = nc.dram_tensor("wv", [C, DQ], BF, kind="ExternalInput").ap()
    wp = nc.dram_tensor("wp", [DQ, C], BF, kind="ExternalInput").ap()
    bqk = nc.dram_tensor("bqk", [128, 4], F32, kind="ExternalInput").ap()
    bvp = nc.dram_tensor("bvp", [128, 2], F32, kind="ExternalInput").ap()
    # bf16 output, stored as the two 128-channel contraction halves of the
    # projection; the host sums them (and the core partials) in f32.  Adds
    # ~0.4% rounding per partial, well inside the error budget.
    out = nc.dram_tensor("out_p", [2, nt, C], BF, kind="ExternalOutput").ap()

    with tile.TileContext(nc) as tc:
        with (
            tc.tile_pool(name="persist", bufs=1) as persist,
            tc.tile_pool(name="pt_pool", bufs=20) as pt_pool,
            tc.tile_pool(name="stage", bufs=6) as stage,
            tc.tile_pool(name="osb_pool", bufs=2) as osb_pool,
            tc.tile_pool(name="rc_pool", bufs=2) as rc_pool,
            tc.tile_pool(name="ps_qkv", bufs=2, space="PSUM") as ps_qkv,
            tc.tile_pool(name="ps_s", bufs=2, space="PSUM") as ps_s,
            tc.tile_pool(name="ps_o", bufs=1, space="PSUM") as ps_o,
        ):
            # ---------------- persistent SBUF state ----------------
            # load order matters: wk + xT feed the first K^T chains; wv/wp
            # are only needed once attention is underway.
            # x^T lives in TWO tiles split at token 512: the tile framework
            # approximates write regions as bounding byte-ranges, so a late
            # chunk write like [po 0:4, 512:1024] would falsely alias the
            # first block's [po 1, 0:512] reads and stall the q00 chain on
            # DMAs it doesn't need.  A separate first-block tile kills that.
            xTa_sb = persist.tile([128, 8, min(nt, 512)], BF)
            if nt > 512:
                xTb_sb = persist.tile([128, 8, nt - 512], BF, name="xTb_sb")
            else:
                xTb_sb = None

            def xT_ap(ci, lo, hi):
                # [128, hi-lo] view of x^T chunk ci, tokens [lo, hi)
                if hi <= 512:
                    return xTa_sb[:, ci, lo:hi]
                assert lo >= 512
                return xTb_sb[:, ci, lo - 512:hi - 512]
            wq_sb = persist.tile([128, 8, DQ], BF)
            wk_sb = persist.tile([128, 8, DQ], BF)
            wv_sb = persist.tile([128, 8, DQ], BF)
            bqk_sb = persist.tile([128, 4], F32)
            bvp_sb = persist.tile([128, 2], F32)
            wp_sb = persist.tile([128, 2, C], BF)
            # Few big DMA instructions (the ~1.3us sequencer issue cost per
            # DMA dominates; transfers run on 16 parallel DMA engines).
            # x^T rides the SP hardware queue in 512-token chunks so the first
            # K/Q chains start early; weights ride the idle Pool (SWDGE) queue.
            xT3 = xT.rearrange("(po pi) n -> pi po n", pi=128)
            wk3 = wk.rearrange("(po pi) c -> pi po c", pi=128)
            wq3 = wq.rearrange("(po pi) c -> pi po c", pi=128)
            wv3 = wv.rearrange("(po pi) c -> pi po c", pi=128)
            wp3 = wp.rearrange("(po pi) c -> pi po c", pi=128)
            def _xt(t):
                if t == 0:
                    return
                sl = slice(t * 512, (t + 1) * 512)
                slb = slice(t * 512 - 512, (t + 1) * 512 - 512)
                nc.sync.dma_start(xTb_sb[:, 0:4, slb], xT3[:, 0:4, sl])
                nc.sync.dma_start(xTb_sb[:, 4:8, slb], xT3[:, 4:8, sl])

            ones_sb = persist.tile([1, 512], BF)
            nc.vector.memset(ones_sb[:], 1.0)
            # warm the PE clock (HAM) immediately: a few tiny matmuls up
            # front get the clock ramp going while the first DMAs land
            warm_ps = ps_qkv.tile([128, 512], F32, tag="qkv", name="warm_ps")
            for i in range(8):
                nc.tensor.matmul(warm_ps[:, 0:256], ones_sb[:, 0:128],
                                 ones_sb[:, 0:256], start=(i == 0),
                                 stop=(i == 7), skip_group_check=True)
            # NOTE: no SBUF read-back of warm_ps: a DVE copy here would wait
            # for the last bridge warm inside the k00 chain and head-of-line
            # block the DVE queue ahead of the k00/q00 bias adds, delaying
            # the first exp by ~2.5us.

            # first weight/token chunks spread across FOUR issue queues (SP,
            # Pool/SWDGE, DVE, ACT) so the first K-chain's inputs all land by
            # ~2.5us instead of trickling off one serialized SP queue
            # (565ns/issue).  Tokens 0-511 must be complete in ALL 8
            # contraction chunks before the first chains' late matmuls.
            # The SWDGE (gpsimd) path transfers on its own track, adding
            # bandwidth in parallel with the HWDGE pool: it carries part of
            # the critical first token block plus wv and the late token
            # chunks, so the HWDGE pool can focus on weights + early tokens.
            # bqk rides the SWDGE queue FIRST (it gates the k00/q00 bias
            # adds); the first token block's xT streams on SWDGE in parallel
            # with the weights on HWDGE, so wq[4:8] (the last input of the
            # q00 chain) lands by ~5us instead of ~6.5us.
            nc.sync.dma_start(wk_sb[:, 0:4, :], wk3[:, 0:4, :])
            nc.gpsimd.dma_start(xTa_sb[:, 0:2, :], xT3[:, 0:2, 0:512])
            nc.scalar.dma_start(xTa_sb[:, 2:4, :], xT3[:, 2:4, 0:512])
            nc.sync.dma_start(wk_sb[:, 4:8, :], wk3[:, 4:8, :])
            nc.scalar.dma_start(xTa_sb[:, 4:6, :], xT3[:, 4:6, 0:512])
            nc.gpsimd.dma_start(bqk_sb[:], bqk)
            nc.sync.dma_start(xTa_sb[:, 6:8, :], xT3[:, 6:8, 0:512])
            nc.sync.dma_start(wq_sb[:, 0:4, :], wq3[:, 0:4, :])
            nc.sync.dma_start(wq_sb[:, 4:8, :], wq3[:, 4:8, :])
            # wv AFTER wq on the same queue: an early SWDGE-issued wv slots
            # its 512KB transfer into the shared DMA track ahead of wq and
            # delays the q00 chain (and the first exp) by ~2us.
            nc.sync.dma_start(wv_sb[:], wv3)
            for t in range(1, n_tc):
                _xt(t)
                if t == 1:
                    nc.sync.dma_start(bvp_sb[:], bvp)
                elif t == 2:
                    nc.sync.dma_start(wp_sb[:], wp3)
            if n_tc < 3:
                nc.sync.dma_start(bvp_sb[:], bvp)
                nc.sync.dma_start(wp_sb[:], wp3)
            ident_sb = persist.tile([128, 128], BF)
            masks.make_identity(nc, ident_sb[:])

            qT_sb = [persist.tile([128, nt], BF, tag=f"qT{p}", name=f"qT{p}")
                     for p in range(2)]
            kT_sb = [persist.tile([128, nt], BF, tag=f"kT{p}", name=f"kT{p}")
                     for p in range(2)]
            oT_sb = [persist.tile([128, n_tc, 4, 128], BF, tag=f"oT{p}",
                                  name=f"oT{p}")
                     for p in range(2)]
            # V layout [keys, kt, head, 65]: col 64 of each head is a ones
            # column (preset once) so AV accumulates softmax denominators.
            v_sb = persist.tile([128, n_kt, HPC, D + 1], BF)
            nc.vector.memset(v_sb[:, :, :, 64:65], 1.0)

            # ---------------- QKV chain emitters ----------------
            def qk_chain(w_sb, bcol, dst, p, t, warm_after=None):
                # warm_after={ci: n} interleaves n dummy warm matmuls after
                # chunk ci: during the preamble the chain stalls on DMA
                # arrivals, and any PE idle >100ns drops the clock back to a
                # low p-state; the dummies keep the busy-streak alive so the
                # real matmuls stay at full speed.
                ps = ps_qkv.tile([128, 512], F32, tag="qkv")
                for ci in range(8):
                    nc.tensor.matmul(
                        ps[:, :],
                        w_sb[:, ci, p * 128:(p + 1) * 128],
                        xT_ap(ci, t * 512, (t + 1) * 512),
                        start=(ci == 0), stop=(ci == 7))
                    if warm_after and ci in warm_after:
                        for _ in range(warm_after[ci]):
                            nc.tensor.matmul(
                                warm_ps[:, 0:256], ones_sb[:, 0:128],
                                ones_sb[:, 0:256], start=True, stop=True,
                                skip_group_check=True)
                nc.vector.tensor_scalar_add(dst[:, t * 512:(t + 1) * 512],
                                            ps[:, :], bqk_sb[:, bcol:bcol + 1])

            def v_chain(tt):
                ps = ps_qkv.tile([128, 8, 64], F32, tag="qkv")
                for ci in range(8):
                    nc.tensor.matmul(
                        ps[:, 0:4, :],
                        xT_ap(ci, tt * 128, (tt + 1) * 128),
                        wv_sb[:, ci, :],
                        start=(ci == 0), stop=(ci == 7))
                # v bias is added after normalization (it commutes through
                # the softmax average), via the oT copy's per-partition add
                nc.vector.tensor_copy(v_sb[:, tt, :, 0:64], ps[:, 0:4, :])

            def qk_half(w_sb, bcol, dst, p, t, half):
                # 256-token half chain: a finer-grained (853ns) PE insert
                # for pacing the overloaded early blocks
                ps = ps_qkv.tile([128, 256], F32, tag="qkv", name="ps_h")
                lo = t * 512 + half * 256
                for ci in range(8):
                    nc.tensor.matmul(
                        ps[:, :],
                        w_sb[:, ci, p * 128:(p + 1) * 128],
                        xT_ap(ci, lo, lo + 256),
                        start=(ci == 0), stop=(ci == 7))
                nc.vector.tensor_scalar_add(dst[:, lo:lo + 256], ps[:, :],
                                            bqk_sb[:, bcol:bcol + 1])

            # bqk_sb columns: 0,1 = q bias pair 0/1; 2,3 = k bias pair 0/1
            def k_chain(p, t, warm_after=None):
                qk_chain(wk_sb, 2 + p, kT_sb[p], p, t, warm_after)

            def q_chain(p, t, warm_after=None):
                qk_chain(wq_sb, 0 + p, qT_sb[p], p, t, warm_after)

            def K2(p, t, half):
                return lambda: qk_half(wk_sb, 2 + p, kT_sb[p], p, t, half)

            def Q2(p, t, half):
                return lambda: qk_half(wq_sb, 0 + p, qT_sb[p], p, t, half)

            # Preamble: just enough for the first attention group to start.
            k_chain(0, 0, warm_after={3: 3, 5: 4})
            q_chain(0, 0)

            # Static insert schedule: each producer chain is emitted 2+
            # groups before its first consumer needs it.  K0 chains feed
            # block 0's own key sweep (forced early); V chains feed the
            # lagged AV pops (spread over blocks 0-1); K1/Q1 chains land
            # just-in-time in blocks 2-5 where the PE has slack.
            def V(tt):
                return lambda: v_chain(tt)

            def K(p, t):
                return lambda: k_chain(p, t)

            def Q(p, t):
                return lambda: q_chain(p, t)

            # ---------------- attention + projection ----------------
            out4 = out.rearrange("pp q (a c) -> pp q a c", a=2)

            def make_proj(qt, pp=None, copy=("dve", "dve"), tail_pool=False,
                          store="sync"):
                # Both column halves of a query tile in one task so the
                # store is a single DMA (HWDGE descriptor-gen serializes).
                # pp=None: both contraction halves accumulated (stored in
                # partial slot 0).  For the LAST query chunk only, the two
                # halves are separate partial stores summed on the host:
                # its pp=0 half only needs pair-0's oT, ready a whole sweep
                # early, so the tail's PE work halves.  `copy` picks the
                # PSUM->SBUF copy engine per half ("act"/"dve"/"pool") and
                # `store` the DMA issue queue, so the end-of-kernel tasks
                # can spread across the otherwise-idle engines.
                def _copy(eng, dst, src):
                    if eng == "act":
                        nc.scalar.copy(dst, src)
                    elif eng == "pool":
                        nc.gpsimd.tensor_copy(dst, src)
                    else:
                        nc.vector.tensor_copy(dst, src)

                def proj():
                    ost = stage.tile([128, 2, 512], BF, tag="ost", name="ost")
                    for nh in range(2):
                        pool = ps_s if (tail_pool and nh == 1) else ps_qkv
                        pps = pool.tile(
                            [128, 512], F32,
                            tag="s" if pool is ps_s else "qkv", name="pps")
                        for ppi in ([0, 1] if pp is None else [pp]):
                            nc.tensor.matmul(
                                pps[:, :],
                                oT_sb[ppi][:, qt // 4, qt % 4, :],
                                wp_sb[:, ppi, nh * 512:(nh + 1) * 512],
                                start=(pp is not None or ppi == 0),
                                stop=(pp is not None or ppi == 1))
                        _copy(copy[nh], ost[:, nh, :], pps[:, :])
                    eng = {"swdge": nc.gpsimd, "scalar": nc.scalar,
                           "sync": nc.sync}[store]
                    eng.dma_start(
                        out4[pp or 0, qt * 128:(qt + 1) * 128, :, :],
                        ost[:, :, :])
                return proj

            def make_norm_dve(o_ps, rc_sb, O_sb):
                # normalization multiplies; emitted at the START of the next
                # block so the DVE has them done well before the transposes.
                # One zero-stride-broadcast multiply per head instead of 8
                # per-subtile ops: the DVE per-op overhead dominates there.
                def norm_dve():
                    for hh in range(2):
                        nc.vector.tensor_mul(
                            O_sb[:, :, hh, :],
                            o_ps[hh][:, :, 0:64],
                            rc_sb[:, hh, :, :].broadcast_to([128, 4, 64]))
                return norm_dve

            def make_norm_pe(O_sb, p, qc, last_block):
                # transpose O back to O^T pair layout + oT copy (with the
                # deferred v-bias add); emitted 2 groups into the next block.
                def norm_pe():
                    tr = ps_qkv.tile([128, 4, 128], BF, tag="qkv",
                                     name="tr_ps")
                    for qi in range(4):
                        nc.tensor.transpose(tr[:, qi, :], O_sb[:, qi, :, :],
                                            ident_sb[:, :])
                    nc.vector.tensor_scalar_add(oT_sb[p][:, qc, :, :],
                                                tr[:, :, :],
                                                bvp_sb[:, p:p + 1])
                    # queue projections now that oT[p] is written: fused
                    # tasks for pair-1 chunks; for the last query chunk its
                    # pair-0 partial is queued as soon as the pair-0 sweep
                    # finishes, so the tail only runs its pair-1 half
                    if p == 1 and not last_block:
                        for qt4 in range(4):
                            deferred.append(make_proj(qc * 4 + qt4))
                    elif p == 0 and qc == n_tc - 1:
                        for qt4 in range(4):
                            deferred.append(make_proj(qc * 4 + qt4, 0))
                return norm_pe

            deferred = []
            blocks = [(qc, 0) for qc in range(n_tc)] + \
                     [(qc, 1) for qc in range(n_tc)]
            nb = len(blocks)
            carry = (n_tc == 4 and n_ktg == 8)
            if carry:
                # (block, group) -> producer-chain thunks, each placed 2+
                # groups before its first consumer.
                sched = {
                    (0, 0): [V(0), K2(0, 1, 0)], (0, 1): [K2(0, 1, 1), V(1)],
                    (0, 2): [V(2), K2(0, 2, 0)], (0, 3): [K2(0, 2, 1), V(3)],
                    (0, 4): [V(4), K2(0, 3, 0)], (0, 5): [K2(0, 3, 1), V(5)],
                    (0, 6): [Q(0, 1), V(6)], (0, 7): [V(7), V(8)],
                    (1, 0): [V(9), V(10)], (1, 1): [V(11), V(12)],
                    (1, 2): [V(13), V(14)], (1, 3): [V(15)],
                    (1, 5): [Q(0, 2)],
                    (2, 4): [Q(0, 3)], (2, 5): [K(1, 0)], (2, 6): [Q(1, 0)],
                    (3, 2): [K(1, 1)], (3, 4): [K(1, 2)], (3, 6): [Q(1, 1)],
                    (4, 2): [K(1, 3)], (4, 5): [Q(1, 2)],
                    (5, 5): [Q(1, 3)],
                }

                # AV pops per (block, group).  Steady state: the 9 carried
                # halves of block bi-1 drain at g0-g4; own pops resume after
                # the g5 memset; block bi hands 9 halves to block bi+1.  The
                # last two blocks pop faster so the final block's norm can
                # run at its g3/g4 (freeing its projections early) and the
                # loop ends with every AV half already emitted - only the
                # exp(g7)-gated kt14/kt15 execute after the last exp.
                def pops_for(bi, g):
                    if bi == 0:
                        return (1 if g >= 4 else 0, 1 if g >= 5 else 0)
                    if bi == nb - 1 and g >= 5:
                        return (2, 2)
                    return {0: (1, 1), 1: (1, 1), 2: (1, 1), 3: (1, 1),
                            4: (0, 1), 5: (1, 1), 6: (1, 1), 7: (2, 1)}[g]
            else:
                # small-sim fallback: dense schedule, in-block drain
                sched = {}
                fill = ([V(tt) for tt in range(2 * n_ktg)]
                        + [K(0, t) for t in range(1, n_tc)]
                        + [Q(0, t) for t in range(1, n_tc)]
                        + [K(1, t) for t in range(n_tc)]
                        + [Q(1, t) for t in range(n_tc)])
                gi = 0
                while fill:
                    b, g = divmod(gi % (nb * n_ktg), n_ktg)
                    sched.setdefault((b, g), []).append(fill.pop(0))
                    gi += 1
                j0l = min(2, n_ktg - 1)
                j1l = min(3, n_ktg - 1)

                def pops_for(bi, g):
                    return (1 if g >= j0l else 0, 1 if g >= j1l else 0)

            def make_av_half(ktg, pt, j, o_ps, p):
                # one key tile's 8 AV matmuls (both heads, 4 query sub-tiles)
                def av():
                    kt = ktg * 2 + j
                    for hh in range(2):
                        h = 2 * p + hh
                        for qi in range(4):
                            nc.tensor.matmul(
                                o_ps[hh][:, qi, :],
                                pt[hh][:, j * 512 + qi * 128:
                                       j * 512 + (qi + 1) * 128],
                                v_sb[:, kt, h, :],
                                start=False,
                                stop=(kt == 2 * n_ktg - 1),
                                skip_group_check=True)
                return av

            j0q, j1q = [], []
            pending = None  # previous block's (o_ps, qc, p) awaiting norm
            pend_pe = None
            for bi, (qc, p) in enumerate(blocks):
                o_ps = [ps_o.tile([128, 4, 65], F32, tag=f"o{_h}",
                                  name=f"o_ps{_h}")
                        for _h in range(2)]
                # AV accumulation groups for the 4 query sub-tiles share a
                # PSUM bank; a start=True matmul resets the whole bank, so
                # the bank is memset once (g5, after the prev block's norm
                # reads) and all AV matmuls accumulate with start=False.
                def o_memset(o_ps=o_ps):
                    for _h in range(2):
                        nc.vector.memset(o_ps[_h][:, :, :], 0.0)
                last_block = (bi == nb - 1)
                if bi == 0 or not carry:
                    o_memset()
                for ktg in range(n_ktg):
                    s_ps = [ps_s.tile([128, 1024], F32, tag="s",
                                      name=f"s_ps{_h}")
                            for _h in range(2)]
                    pt = [pt_pool.tile([128, 1024], BF, tag="pt",
                                       name=f"pt{_h}")
                          for _h in range(2)]
                    # hh-major score order: each head's exp is emitted as
                    # soon as that head's two key tiles are scored
                    for hh in range(2):
                        for j in range(2):
                            kt = ktg * 2 + j
                            nc.tensor.matmul(
                                s_ps[hh][:, j * 512:(j + 1) * 512],
                                kT_sb[p][hh * 64:(hh + 1) * 64,
                                         kt * 128:(kt + 1) * 128],
                                qT_sb[p][hh * 64:(hh + 1) * 64,
                                         qc * 512:(qc + 1) * 512],
                                start=True, stop=True)
                        nc.scalar.activation(pt[hh][:, :], s_ps[hh][:, :],
                                             AF.Exp, scale=SCALE)
                    # prev block's normalization: its last carried AV was
                    # emitted at g4 (g2 for the last block), so the
                    # denominators are complete; the memset must follow the
                    # norm's o_ps reads (in-order DVE) and precede this
                    # block's own AV pops.
                    norm_g = 5
                    if carry and pending is not None and ktg == norm_g:
                        po, pqc, pp_ = pending
                        rc_sb = rc_pool.tile([128, 2, 4, 1], F32, tag="rc",
                                             name="rc_sb")
                        O_sb = osb_pool.tile([128, 4, 2, 64], BF, tag="osb",
                                             name="O_sb")
                        for hh in range(2):
                            nc.vector.reciprocal(rc_sb[:, hh, :, :],
                                                 po[hh][:, :, 64:65])
                        make_norm_dve(po, rc_sb, O_sb)()
                        o_memset()
                        pend_pe = make_norm_pe(O_sb, pp_, pqc,
                                               last_block=False)
                        pending = None
                    if carry and pend_pe is not None and ktg == norm_g + 1:
                        pend_pe()
                        pend_pe = None
                    j0q.append(make_av_half(ktg, pt, 0, o_ps, p))
                    j1q.append(make_av_half(ktg, pt, 1, o_ps, p))
                    n0, n1 = pops_for(bi, ktg)
                    for _ in range(n1):
                        if j1q:
                            j1q.pop(0)()
                    for _ in range(n0):
                        if j0q:
                            j0q.pop(0)()
                    ins = sched.get((bi, ktg), [])
                    for th in ins:
                        th()
                    # projection tasks fill otherwise-empty groups, but stay
                    # out of the last block's final groups (they'd delay the
                    # last exps, which gate the whole tail)
                    if not ins and deferred and not (last_block and ktg >= 5):
                        deferred.pop(0)()
                if not carry:
                    while j0q or j1q:
                        if j1q:
                            j1q.pop(0)()
                        if j0q:
                            j0q.pop(0)()
                    if pending is not None:
                        po, pqc, pp_ = pending
                        rc_sb = rc_pool.tile([128, 2, 4, 1], F32, tag="rc",
                                             name="rc_sb")
                        O_sb = osb_pool.tile([128, 4, 2, 64], BF, tag="osb",
                                             name="O_sb")
                        for hh in range(2):
                            nc.vector.reciprocal(rc_sb[:, hh, :, :],
                                                 po[hh][:, :, 64:65])
                        make_norm_dve(po, rc_sb, O_sb)()
                        make_norm_pe(O_sb, pp_, pqc, last_block=False)()
                        pending = None
                pending = (o_ps, qc, p)
            # drain the last block's carried AV halves (4 in the carry
            # schedule); kt14 (j0) then kt15 (j1) land last so the stop
            # flag closes the accumulation group.
            while j0q or j1q:
                if j0q:
                    j0q.pop(0)()
                if j1q:
                    j1q.pop(0)()
            lb_o_ps, lb_qc, _lb_p = pending
            lb_rc = rc_pool.tile([128, 2, 4, 1], F32, tag="rc", name="lb_rc")
            lb_O = osb_pool.tile([128, 4, 2, 64], BF, tag="osb", name="lb_O")
            for hh in range(2):
                nc.vector.reciprocal(lb_rc[:, hh, :, :],
                                     lb_o_ps[hh][:, :, 64:65])
            # tail: last block's normalization; scales prefetched onto the
            # DVE while the PE chews leftover deferred projections, then a
            # hand-interleaved transpose/copy/projection pipeline.
            if deferred:
                deferred.pop(0)()
            # qi0's scales first so the first transpose (and the whole
            # per-qi tail pipeline behind it) starts half an op earlier
            for hh in range(2):
                nc.vector.tensor_scalar_mul(
                    lb_O[:, 0, hh, :], lb_o_ps[hh][:, 0, 0:64],
                    lb_rc[:, hh, 0, :])
            for hh in range(2):
                nc.vector.tensor_mul(
                    lb_O[:, 1:4, hh, :],
                    lb_o_ps[hh][:, 1:4, 0:64],
                    lb_rc[:, hh, 1:4, :].broadcast_to([128, 3, 64]))
            while deferred:
                deferred.pop(0)()
            tr_t = ps_qkv.tile([128, 4, 128], BF, tag="qkv", name="tr_ps_t")

            def tail_tr(qi):
                nc.tensor.matmul(tr_t[:, qi, :], lb_O[:, qi, :, :],
                                 ident_sb[:, :], is_transpose=True,
                                 skip_group_check=True)
                nc.vector.tensor_scalar_add(oT_sb[1][:, lb_qc, qi, :],
                                            tr_t[:, qi, :],
                                            bvp_sb[:, 1:2])

            tail_tr(0)
            tail_tr(1)
            for qi in range(4):
                qt = lb_qc * 4 + qi
                if qi + 2 < 4:
                    tail_tr(qi + 2)
                make_proj(qt, 1, copy=("act", "dve"), tail_pool=True)()

    nc.finalize()
    return nc


def make_core_inputs(x, W_qkv, b_qkv, W_proj, nt=NT):
    """Host-side shard prep: returns in_maps list for the 8 cores."""
    in_maps = []
    for core in range(NCORES):
        b, g = divmod(core, NCORES // B)
        lo, hi = g * DQ, (g + 1) * DQ
        xTb = np.ascontiguousarray(x[b].T).astype(BF16)
        wq_c = np.ascontiguousarray(W_qkv[:, lo:hi]).astype(BF16)
        wk_c = np.ascontiguousarray(W_qkv[:, C + lo:C + hi]).astype(BF16)
        wv_c = np.ascontiguousarray(W_qkv[:, 2 * C + lo:2 * C + hi]).astype(BF16)
        bvp_c = np.stack([
            b_qkv[2 * C + lo:2 * C + lo + 128],
            b_qkv[2 * C + lo + 128:2 * C + hi],
        ], axis=1).astype(np.float32)
        wp_c = np.ascontiguousarray(W_proj[lo:hi, :]).astype(BF16)
        bqk_c = np.stack([
            b_qkv[lo:lo + 128], b_qkv[lo + 128:hi],
            b_qkv[C + lo:C + lo + 128], b_qkv[C + lo + 128:C + hi],
        ], axis=1).astype(np.float32)
        in_maps.append({
            "xT": xTb[:, :nt].copy(), "wq": wq_c, "wk": wk_c, "wv": wv_c,
            "wp": wp_c, "bqk": bqk_c, "bvp": bvp_c,
        })
    return in_maps


_prog_cache = {}


def _get_program(nt=NT):
    if nt not in _prog_cache:
        _prog_cache[nt] = build_program(nt)
    return _prog_cache[nt]


def kernel(x, W_qkv, b_qkv, W_proj, b_proj, _run_kwargs=None):
    x = np.asarray(x, dtype=np.float32)
    W_qkv = np.asarray(W_qkv, dtype=np.float32)
    b_qkv = np.asarray(b_qkv, dtype=np.float32)
    W_proj = np.asarray(W_proj, dtype=np.float32)
    b_proj = np.asarray(b_proj, dtype=np.float32)

    nc = _get_program()
    in_maps = make_core_inputs(x, W_qkv, b_qkv, W_proj)
    for attempt in range(3):
        res = run_bass_kernel_spmd(nc, in_maps, core_ids=list(range(NCORES)),
                                   **(_run_kwargs or {}))
        out = np.zeros((B, NT, C), dtype=np.float32)
        for core in range(NCORES):
            b = core // (NCORES // B)
            part = np.asarray(res.results[core]["out_p"], dtype=np.float32)
            out[b] += part[0]
            # only the last query chunk is stored as two separate halves
            out[b, (NT // 4) * 3:] += part[1][(NT // 4) * 3:]
        if np.isfinite(out).all():
            break
        # transient device flake (observed rarely under axon): retry
    out += b_proj[None, None, :]
    if _run_kwargs:
        kernel.last_results = res
    return out



# revision 64
# speedup vs baseline: 1.0010x; 1.0002x over previous
"""Trainium2 Bass kernel for multi-head self-attention (B=2, N=2048, C=1024, H=16, d=64).

Sharding: 8 cores = 2 batches x 4 head-groups (4 heads each). Each core computes
QKV for its heads (column-sliced W_qkv), full attention over its heads, and a
row-sliced partial of the output projection. Host sums the 4 partials per batch
and adds b_proj.

Device dataflow (per core, all matmuls bf16 with fp32 PSUM accumulation):
  - x^T is loaded [C, N] so Q^T/K^T come out as [head*d, N] (d on partitions),
    which is exactly the lhsT/rhs layout the scores matmul wants.
  - S^T tile [128 keys, 512 queries] = (K^T chunk)^T-matmul(Q^T chunk), K=64
    contraction; the two heads of a pair sit at partition offsets 0/64.
  - softmax skips the max-subtraction (scores are ~N(0,1); exp is safe in fp32)
    so exp(scale*S) is a single ACT pass straight out of PSUM, cast to bf16.
    The exp stream is the second-busiest engine (~133us); scores for group
    k+1 recycle the PSUM banks the exp of group k reads, so per-head exps are
    emitted as soon as that head's two key tiles are scored.
  - AV runs in O[query, d] orientation: lhsT = P^T slice [128 keys, 128
    queries], rhs = V[128 keys, 65] (65th column ones), so each matmul uses
    all 128 output partitions and costs only 65 output rows - half the PE
    time of the O^T orientation.  Column 64 of the accumulator collects the
    softmax denominator on each query's partition.  The four query sub-tiles
    of a block share a PSUM bank, and a start=True matmul clobbers the whole
    bank, so the bank is memset once and all AV matmuls accumulate with
    start=False.  The two key tiles of a group are emitted in different
    groups (lags 3/5) so region re-writes land clear of their semaphores.
  - normalization is per-partition: one reciprocal [128, 2, 4, 1] per head,
    then one zero-stride-broadcast tensor multiply per head (DVE per-op
    overhead dominates smaller ops) - no broadcast matmuls.
  - O[q, 2*64] flips back to O^T pair layout [128, q] with one PE
    identity-transpose per 128-query tile; the PSUM->SBUF copy doubles as
    the V-bias add (b_v commutes through the softmax average).
  - projection: both 512-column halves of a query tile in one deferred task
    ending in a single bf16 store (HWDGE descriptor-gen serializes, so fewer
    bigger stores win); the host sums partials in f32.  The LAST query
    chunk's two contraction halves are separate partial stores: its pair-0
    half runs a whole sweep early, halving the tail's PE work.
  - scheduling: producer chains drip-feed with hand-tuned pacing (front-
    loaded into each block's early groups, spread so pair-1 blocks aren't
    starved), prev-block normalization splits into a DVE part at group 0 and
    a PE part at group 2, the first input chunks are split fine across three
    DMA issue queues, and the PE is HAM-warmed during the initial DMAs.
"""

import sys

sys.path.insert(0, "/opt/trn_rl_repo")

import numpy as np
import ml_dtypes

import concourse.bass as bass
import concourse.tile as tile
from concourse import bacc, masks, mybir
from concourse.bass_utils import run_bass_kernel_spmd

BF16 = ml_dtypes.bfloat16
F32 = mybir.dt.float32
BF = mybir.dt.bfloat16
AF = mybir.ActivationFunctionType

B, NT, C, H, D = 2, 2048, 1024, 16, 64
NCORES = 8
HPC = 4  # heads per core
DQ = HPC * D  # 256 c_out per q/k/v slice
VW = HPC * (D + 1)  # 260: V with a ones column per head
SCALE = D ** -0.5


def build_program(nt=NT):
    """Build the SPMD Bass program. nt parametrized so a small version can be
    simulated quickly in CoreSim."""
    n_tc = nt // 512  # 512-token chunks
    n_kt = nt // 128  # 128-key tiles
    n_ktg = nt // 256  # groups of 2 key tiles (one exp per head per 1024 cols)

    nc = bacc.Bacc("TRN2", target_bir_lowering=False, debug=False,
                   num_devices=NCORES)

    xT = nc.dram_tensor("xT", [C, nt], BF, kind="ExternalInput").ap()
    wq = nc.dram_tensor("wq", [C, DQ], BF, kind="ExternalInput").ap()
    wk = nc.dram_tensor("wk", [C, DQ], BF, kind="ExternalInput").ap()
    wv = nc.dram_tensor("wv", [C, DQ], BF, kind="ExternalInput").ap()
    wp = nc.dram_tensor("wp", [DQ, C], BF, kind="ExternalInput").ap()
    bqk = nc.dram_tensor("bqk", [128, 4], F32, kind="ExternalInput").ap()
    bvp = nc.dram_tensor("bvp", [128, 2], F32, kind="ExternalInput").ap()
    # bf16 output, stored as the two 128-channel contraction halves of the
    # projection; the host sums them (and the core partials) in f32.  Adds
    # ~0.4% rounding per partial, well inside the error budget.
    out = nc.dram_tensor("out_p", [2, nt, C], BF, kind="ExternalOutput").ap()

    with tile.TileContext(nc) as tc:
        with (
            tc.tile_pool(name="persist", bufs=1) as persist,
            tc.tile_pool(name="pt_pool", bufs=20) as pt_pool,
            tc.tile_pool(name="stage", bufs=6) as stage,
            tc.tile_pool(name="osb_pool", bufs=2) as osb_pool,
            tc.tile_pool(name="rc_pool", bufs=2) as rc_pool,
            tc.tile_pool(name="ps_qkv", bufs=2, space="PSUM") as ps_qkv,
            tc.tile_pool(name="ps_s", bufs=2, space="PSUM") as ps_s,
            tc.tile_pool(name="ps_o", bufs=1, space="PSUM") as ps_o,
        ):
            # ---------------- persistent SBUF state ----------------
            # load order matters: wk + xT feed the first K^T chains; wv/wp
            # are only needed once attention is underway.
            # x^T lives in TWO tiles split at token 512: the tile framework
            # approximates write regions as bounding byte-ranges, so a late
            # chunk write like [po 0:4, 512:1024] would falsely alias the
            # first block's [po 1, 0:512] reads and stall the q00 chain on
            # DMAs it doesn't need.  A separate first-block tile kills that.
            xTa_sb = persist.tile([128, 8, min(nt, 512)], BF)
            if nt > 512:
                xTb_sb = persist.tile([128, 8, nt - 512], BF, name="xTb_sb")
            else:
                xTb_sb = None

            def xT_ap(ci, lo, hi):
                # [128, hi-lo] view of x^T chunk ci, tokens [lo, hi)
                if hi <= 512:
                    return xTa_sb[:, ci, lo:hi]
                assert lo >= 512
                return xTb_sb[:, ci, lo - 512:hi - 512]
            wq_sb = persist.tile([128, 8, DQ], BF)
            wk_sb = persist.tile([128, 8, DQ], BF)
            wv_sb = persist.tile([128, 8, DQ], BF)
            bqk_sb = persist.tile([128, 4], F32)
            bvp_sb = persist.tile([128, 2], F32)
            wp_sb = persist.tile([128, 2, C], BF)
            # Few big DMA instructions (the ~1.3us sequencer issue cost per
            # DMA dominates; transfers run on 16 parallel DMA engines).
            # x^T rides the SP hardware queue in 512-token chunks so the first
            # K/Q chains start early; weights ride the idle Pool (SWDGE) queue.
            xT3 = xT.rearrange("(po pi) n -> pi po n", pi=128)
            wk3 = wk.rearrange("(po pi) c -> pi po c", pi=128)
            wq3 = wq.rearrange("(po pi) c -> pi po c", pi=128)
            wv3 = wv.rearrange("(po pi) c -> pi po c", pi=128)
            wp3 = wp.rearrange("(po pi) c -> pi po c", pi=128)
            def _xt(t):
                if t == 0:
                    return
                sl = slice(t * 512, (t + 1) * 512)
                slb = slice(t * 512 - 512, (t + 1) * 512 - 512)
                nc.sync.dma_start(xTb_sb[:, 0:4, slb], xT3[:, 0:4, sl])
                nc.sync.dma_start(xTb_sb[:, 4:8, slb], xT3[:, 4:8, sl])

            ones_sb = persist.tile([1, 512], BF)
            nc.vector.memset(ones_sb[:], 1.0)
            # warm the PE clock (HAM) immediately: a few tiny matmuls up
            # front get the clock ramp going while the first DMAs land
            warm_ps = ps_qkv.tile([128, 512], F32, tag="qkv", name="warm_ps")
            for i in range(8):
                nc.tensor.matmul(warm_ps[:, 0:256], ones_sb[:, 0:128],
                                 ones_sb[:, 0:256], start=(i == 0),
                                 stop=(i == 7), skip_group_check=True)
            # NOTE: no SBUF read-back of warm_ps: a DVE copy here would wait
            # for the last bridge warm inside the k00 chain and head-of-line
            # block the DVE queue ahead of the k00/q00 bias adds, delaying
            # the first exp by ~2.5us.

            # first weight/token chunks spread across FOUR issue queues (SP,
            # Pool/SWDGE, DVE, ACT) so the first K-chain's inputs all land by
            # ~2.5us instead of trickling off one serialized SP queue
            # (565ns/issue).  Tokens 0-511 must be complete in ALL 8
            # contraction chunks before the first chains' late matmuls.
            # The SWDGE (gpsimd) path transfers on its own track, adding
            # bandwidth in parallel with the HWDGE pool: it carries part of
            # the critical first token block plus wv and the late token
            # chunks, so the HWDGE pool can focus on weights + early tokens.
            # bqk rides the SWDGE queue FIRST (it gates the k00/q00 bias
            # adds); the first token block's xT streams on SWDGE in parallel
            # with the weights on HWDGE, so wq[4:8] (the last input of the
            # q00 chain) lands by ~5us instead of ~6.5us.
            nc.sync.dma_start(wk_sb[:, 0:4, :], wk3[:, 0:4, :])
            nc.gpsimd.dma_start(xTa_sb[:, 0:2, :], xT3[:, 0:2, 0:512])
            nc.scalar.dma_start(xTa_sb[:, 2:4, :], xT3[:, 2:4, 0:512])
            nc.sync.dma_start(wk_sb[:, 4:8, :], wk3[:, 4:8, :])
            nc.scalar.dma_start(xTa_sb[:, 4:6, :], xT3[:, 4:6, 0:512])
            nc.gpsimd.dma_start(bqk_sb[:], bqk)
            nc.sync.dma_start(xTa_sb[:, 6:8, :], xT3[:, 6:8, 0:512])
            nc.sync.dma_start(wq_sb[:, 0:4, :], wq3[:, 0:4, :])
            nc.sync.dma_start(wq_sb[:, 4:8, :], wq3[:, 4:8, :])
            # wv AFTER wq on the same queue: an early SWDGE-issued wv slots
            # its 512KB transfer into the shared DMA track ahead of wq and
            # delays the q00 chain (and the first exp) by ~2us.
            nc.sync.dma_start(wv_sb[:], wv3)
            for t in range(1, n_tc):
                _xt(t)
                if t == 1:
                    nc.sync.dma_start(bvp_sb[:], bvp)
                elif t == 2:
                    nc.sync.dma_start(wp_sb[:], wp3)
            if n_tc < 3:
                nc.sync.dma_start(bvp_sb[:], bvp)
                nc.sync.dma_start(wp_sb[:], wp3)
            ident_sb = persist.tile([128, 128], BF)
            masks.make_identity(nc, ident_sb[:])

            qT_sb = [persist.tile([128, nt], BF, tag=f"qT{p}", name=f"qT{p}")
                     for p in range(2)]
            kT_sb = [persist.tile([128, nt], BF, tag=f"kT{p}", name=f"kT{p}")
                     for p in range(2)]
            oT_sb = [persist.tile([128, n_tc, 4, 128], BF, tag=f"oT{p}",
                                  name=f"oT{p}")
                     for p in range(2)]
            # V layout [keys, kt, head, 65]: col 64 of each head is a ones
            # column (preset once) so AV accumulates softmax denominators.
            v_sb = persist.tile([128, n_kt, HPC, D + 1], BF)
            nc.vector.memset(v_sb[:, :, :, 64:65], 1.0)

            # ---------------- QKV chain emitters ----------------
            def qk_chain(w_sb, bcol, dst, p, t, warm_after=None):
                # warm_after={ci: n} interleaves n dummy warm matmuls after
                # chunk ci: during the preamble the chain stalls on DMA
                # arrivals, and any PE idle >100ns drops the clock back to a
                # low p-state; the dummies keep the busy-streak alive so the
                # real matmuls stay at full speed.
                ps = ps_qkv.tile([128, 512], F32, tag="qkv")
                for ci in range(8):
                    nc.tensor.matmul(
                        ps[:, :],
                        w_sb[:, ci, p * 128:(p + 1) * 128],
                        xT_ap(ci, t * 512, (t + 1) * 512),
                        start=(ci == 0), stop=(ci == 7))
                    if warm_after and ci in warm_after:
                        for _ in range(warm_after[ci]):
                            nc.tensor.matmul(
                                warm_ps[:, 0:256], ones_sb[:, 0:128],
                                ones_sb[:, 0:256], start=True, stop=True,
                                skip_group_check=True)
                nc.vector.tensor_scalar_add(dst[:, t * 512:(t + 1) * 512],
                                            ps[:, :], bqk_sb[:, bcol:bcol + 1])

            def v_chain(tt):
                ps = ps_qkv.tile([128, 8, 64], F32, tag="qkv")
                for ci in range(8):
                    nc.tensor.matmul(
                        ps[:, 0:4, :],
                        xT_ap(ci, tt * 128, (tt + 1) * 128),
                        wv_sb[:, ci, :],
                        start=(ci == 0), stop=(ci == 7))
                # v bias is added after normalization (it commutes through
                # the softmax average), via the oT copy's per-partition add
                nc.vector.tensor_copy(v_sb[:, tt, :, 0:64], ps[:, 0:4, :])

            def qk_half(w_sb, bcol, dst, p, t, half):
                # 256-token half chain: a finer-grained (853ns) PE insert
                # for pacing the overloaded early blocks
                ps = ps_qkv.tile([128, 256], F32, tag="qkv", name="ps_h")
                lo = t * 512 + half * 256
                for ci in range(8):
                    nc.tensor.matmul(
                        ps[:, :],
                        w_sb[:, ci, p * 128:(p + 1) * 128],
                        xT_ap(ci, lo, lo + 256),
                        start=(ci == 0), stop=(ci == 7))
                nc.vector.tensor_scalar_add(dst[:, lo:lo + 256], ps[:, :],
                                            bqk_sb[:, bcol:bcol + 1])

            # bqk_sb columns: 0,1 = q bias pair 0/1; 2,3 = k bias pair 0/1
            def k_chain(p, t, warm_after=None):
                qk_chain(wk_sb, 2 + p, kT_sb[p], p, t, warm_after)

            def q_chain(p, t, warm_after=None):
                qk_chain(wq_sb, 0 + p, qT_sb[p], p, t, warm_after)

            def K2(p, t, half):
                return lambda: qk_half(wk_sb, 2 + p, kT_sb[p], p, t, half)

            def Q2(p, t, half):
                return lambda: qk_half(wq_sb, 0 + p, qT_sb[p], p, t, half)

            # Preamble: just enough for the first attention group to start.
            k_chain(0, 0, warm_after={3: 3, 5: 4})
            q_chain(0, 0)

            # Static insert schedule: each producer chain is emitted 2+
            # groups before its first consumer needs it.  K0 chains feed
            # block 0's own key sweep (forced early); V chains feed the
            # lagged AV pops (spread over blocks 0-1); K1/Q1 chains land
            # just-in-time in blocks 2-5 where the PE has slack.
            def V(tt):
                return lambda: v_chain(tt)

            def K(p, t):
                return lambda: k_chain(p, t)

            def Q(p, t):
                return lambda: q_chain(p, t)

            # ---------------- attention + projection ----------------
            out4 = out.rearrange("pp q (a c) -> pp q a c", a=2)

            def make_proj(qt, pp=None, copy=("dve", "dve"), tail_pool=False,
                          store="sync"):
                # Both column halves of a query tile in one task so the
                # store is a single DMA (HWDGE descriptor-gen serializes).
                # pp=None: both contraction halves accumulated (stored in
                # partial slot 0).  For the LAST query chunk only, the two
                # halves are separate partial stores summed on the host:
                # its pp=0 half only needs pair-0's oT, ready a whole sweep
                # early, so the tail's PE work halves.  `copy` picks the
                # PSUM->SBUF copy engine per half ("act"/"dve"/"pool") and
                # `store` the DMA issue queue, so the end-of-kernel tasks
                # can spread across the otherwise-idle engines.
                def _copy(eng, dst, src):
                    if eng == "act":
                        nc.scalar.copy(dst, src)
                    elif eng == "pool":
                        nc.gpsimd.tensor_copy(dst, src)
                    else:
                        nc.vector.tensor_copy(dst, src)

                def proj():
                    ost = stage.tile([128, 2, 512], BF, tag="ost", name="ost")
                    for nh in range(2):
                        pool = ps_s if (tail_pool and nh == 1) else ps_qkv
                        pps = pool.tile(
                            [128, 512], F32,
                            tag="s" if pool is ps_s else "qkv", name="pps")
                        for ppi in ([0, 1] if pp is None else [pp]):
                            nc.tensor.matmul(
                                pps[:, :],
                                oT_sb[ppi][:, qt // 4, qt % 4, :],
                                wp_sb[:, ppi, nh * 512:(nh + 1) * 512],
                                start=(pp is not None or ppi == 0),
                                stop=(pp is not None or ppi == 1))
                        _copy(copy[nh], ost[:, nh, :], pps[:, :])
                    eng = {"swdge": nc.gpsimd, "scalar": nc.scalar,
                           "sync": nc.sync}[store]
                    eng.dma_start(
                        out4[pp or 0, qt * 128:(qt + 1) * 128, :, :],
                        ost[:, :, :])
                return proj

            def make_norm_dve(o_ps, rc_sb, O_sb):
                # normalization multiplies; emitted at the START of the next
                # block so the DVE has them done well before the transposes.
                # One zero-stride-broadcast multiply per head instead of 8
                # per-subtile ops: the DVE per-op overhead dominates there.
                def norm_dve():
                    for hh in range(2):
                        nc.vector.tensor_mul(
                            O_sb[:, :, hh, :],
                            o_ps[hh][:, :, 0:64],
                            rc_sb[:, hh, :, :].broadcast_to([128, 4, 64]))
                return norm_dve

            def make_norm_pe(O_sb, p, qc, last_block):
                # transpose O back to O^T pair layout + oT copy (with the
                # deferred v-bias add); emitted 2 groups into the next block.
                def norm_pe():
                    tr = ps_qkv.tile([128, 4, 128], BF, tag="qkv",
                                     name="tr_ps")
                    for qi in range(4):
                        nc.tensor.transpose(tr[:, qi, :], O_sb[:, qi, :, :],
                                            ident_sb[:, :])
                    nc.vector.tensor_scalar_add(oT_sb[p][:, qc, :, :],
                                                tr[:, :, :],
                                                bvp_sb[:, p:p + 1])
                    # queue projections now that oT[p] is written: fused
                    # tasks for pair-1 chunks; for the last query chunk its
                    # pair-0 partial is queued as soon as the pair-0 sweep
                    # finishes, so the tail only runs its pair-1 half
                    if p == 1 and not last_block:
                        for qt4 in range(4):
                            deferred.append(make_proj(qc * 4 + qt4))
                    elif p == 0 and qc == n_tc - 1:
                        for qt4 in range(4):
                            deferred.append(make_proj(qc * 4 + qt4, 0))
                return norm_pe

            deferred = []
            blocks = [(qc, 0) for qc in range(n_tc)] + \
                     [(qc, 1) for qc in range(n_tc)]
            nb = len(blocks)
            carry = (n_tc == 4 and n_ktg == 8)
            if carry:
                # (block, group) -> producer-chain thunks, each placed 2+
                # groups before its first consumer.
                sched = {
                    (0, 0): [V(0), K2(0, 1, 0)], (0, 1): [K2(0, 1, 1), V(1)],
                    (0, 2): [V(2), K2(0, 2, 0)], (0, 3): [K2(0, 2, 1), V(3)],
                    (0, 4): [V(4), K2(0, 3, 0)], (0, 5): [K2(0, 3, 1), V(5)],
                    (0, 6): [Q(0, 1), V(6)], (0, 7): [V(7), V(8)],
                    (1, 0): [V(9), V(10)], (1, 1): [V(11), V(12)],
                    (1, 2): [V(13), V(14)], (1, 3): [V(15)],
                    (1, 5): [Q(0, 2)],
                    (2, 4): [Q(0, 3)], (2, 5): [K(1, 0)], (2, 6): [Q(1, 0)],
                    (3, 2): [K(1, 1)], (3, 4): [K(1, 2)], (3, 6): [Q(1, 1)],
                    (4, 2): [K(1, 3)], (4, 5): [Q(1, 2)],
                    (5, 5): [Q(1, 3)],
                }

                # AV pops per (block, group).  Steady state: the 9 carried
                # halves of block bi-1 drain at g0-g4; own pops resume after
                # the g5 memset; block bi hands 9 halves to block bi+1.  The
                # last two blocks pop faster so the final block's norm can
                # run at its g3/g4 (freeing its projections early) and the
                # loop ends with every AV half already emitted - only the
                # exp(g7)-gated kt14/kt15 execute after the last exp.
                def pops_for(bi, g):
                    if bi == 0:
                        return (1 if g >= 4 else 0, 1 if g >= 5 else 0)
                    if bi == nb - 1 and g >= 5:
                        return (2, 2)
                    return {0: (1, 1), 1: (1, 1), 2: (1, 1), 3: (1, 1),
                            4: (0, 1), 5: (1, 1), 6: (1, 1), 7: (2, 1)}[g]
            else:
                # small-sim fallback: dense schedule, in-block drain
                sched = {}
                fill = ([V(tt) for tt in range(2 * n_ktg)]
                        + [K(0, t) for t in range(1, n_tc)]
                        + [Q(0, t) for t in range(1, n_tc)]
                        + [K(1, t) for t in range(n_tc)]
                        + [Q(1, t) for t in range(n_tc)])
                gi = 0
                while fill:
                    b, g = divmod(gi % (nb * n_ktg), n_ktg)
                    sched.setdefault((b, g), []).append(fill.pop(0))
                    gi += 1
                j0l = min(2, n_ktg - 1)
                j1l = min(3, n_ktg - 1)

                def pops_for(bi, g):
                    return (1 if g >= j0l else 0, 1 if g >= j1l else 0)

            def make_av_half(ktg, pt, j, o_ps, p):
                # one key tile's 8 AV matmuls (both heads, 4 query sub-tiles)
                def av():
                    kt = ktg * 2 + j
                    for hh in range(2):
                        h = 2 * p + hh
                        for qi in range(4):
                            nc.tensor.matmul(
                                o_ps[hh][:, qi, :],
                                pt[hh][:, j * 512 + qi * 128:
                                       j * 512 + (qi + 1) * 128],
                                v_sb[:, kt, h, :],
                                start=False,
                                stop=(kt == 2 * n_ktg - 1),
                                skip_group_check=True)
                return av

            j0q, j1q = [], []
            pending = None  # previous block's (o_ps, qc, p) awaiting norm
            pend_pe = None
            for bi, (qc, p) in enumerate(blocks):
                o_ps = [ps_o.tile([128, 4, 65], F32, tag=f"o{_h}",
                                  name=f"o_ps{_h}")
                        for _h in range(2)]
                # AV accumulation groups for the 4 query sub-tiles share a
                # PSUM bank; a start=True matmul resets the whole bank, so
                # the bank is memset once (g5, after the prev block's norm
                # reads) and all AV matmuls accumulate with start=False.
                def o_memset(o_ps=o_ps):
                    for _h in range(2):
                        nc.vector.memset(o_ps[_h][:, :, :], 0.0)
                last_block = (bi == nb - 1)
                if bi == 0 or not carry:
                    o_memset()
                for ktg in range(n_ktg):
                    s_ps = [ps_s.tile([128, 1024], F32, tag="s",
                                      name=f"s_ps{_h}")
                            for _h in range(2)]
                    pt = [pt_pool.tile([128, 1024], BF, tag="pt",
                                       name=f"pt{_h}")
                          for _h in range(2)]
                    # hh-major score order: each head's exp is emitted as
                    # soon as that head's two key tiles are scored
                    for hh in range(2):
                        for j in range(2):
                            kt = ktg * 2 + j
                            nc.tensor.matmul(
                                s_ps[hh][:, j * 512:(j + 1) * 512],
                                kT_sb[p][hh * 64:(hh + 1) * 64,
                                         kt * 128:(kt + 1) * 128],
                                qT_sb[p][hh * 64:(hh + 1) * 64,
                                         qc * 512:(qc + 1) * 512],
                                start=True, stop=True)
                        nc.scalar.activation(pt[hh][:, :], s_ps[hh][:, :],
                                             AF.Exp, scale=SCALE)
                    # prev block's normalization: its last carried AV was
                    # emitted at g4 (g2 for the last block), so the
                    # denominators are complete; the memset must follow the
                    # norm's o_ps reads (in-order DVE) and precede this
                    # block's own AV pops.
                    norm_g = 5
                    if carry and pending is not None and ktg == norm_g:
                        po, pqc, pp_ = pending
                        rc_sb = rc_pool.tile([128, 2, 4, 1], F32, tag="rc",
                                             name="rc_sb")
                        O_sb = osb_pool.tile([128, 4, 2, 64], BF, tag="osb",
                                             name="O_sb")
                        for hh in range(2):
                            nc.vector.reciprocal(rc_sb[:, hh, :, :],
                                                 po[hh][:, :, 64:65])
                        make_norm_dve(po, rc_sb, O_sb)()
                        o_memset()
                        pend_pe = make_norm_pe(O_sb, pp_, pqc,
                                               last_block=False)
                        pending = None
                    if carry and pend_pe is not None and ktg == norm_g + 1:
                        pend_pe()
                        pend_pe = None
                    j0q.append(make_av_half(ktg, pt, 0, o_ps, p))
                    j1q.append(make_av_half(ktg, pt, 1, o_ps, p))
                    n0, n1 = pops_for(bi, ktg)
                    for _ in range(n1):
                        if j1q:
                            j1q.pop(0)()
                    for _ in range(n0):
                        if j0q:
                            j0q.pop(0)()
                    ins = sched.get((bi, ktg), [])
                    for th in ins:
                        th()
                    # projection tasks fill otherwise-empty groups, but stay
                    # out of the last block's final groups (they'd delay the
                    # last exps, which gate the whole tail)
                    if not ins and deferred and not (last_block and ktg >= 5):
                        deferred.pop(0)()
                if not carry:
                    while j0q or j1q:
                        if j1q:
                            j1q.pop(0)()
                        if j0q:
                            j0q.pop(0)()
                    if pending is not None:
                        po, pqc, pp_ = pending
                        rc_sb = rc_pool.tile([128, 2, 4, 1], F32, tag="rc",
                                             name="rc_sb")
                        O_sb = osb_pool.tile([128, 4, 2, 64], BF, tag="osb",
                                             name="O_sb")
                        for hh in range(2):
                            nc.vector.reciprocal(rc_sb[:, hh, :, :],
                                                 po[hh][:, :, 64:65])
                        make_norm_dve(po, rc_sb, O_sb)()
                        make_norm_pe(O_sb, pp_, pqc, last_block=False)()
                        pending = None
                pending = (o_ps, qc, p)
            # drain the last block's carried AV halves (4 in the carry
            # schedule); kt14 (j0) then kt15 (j1) land last so the stop
            # flag closes the accumulation group.
            while j0q or j1q:
                if j0q:
                    j0q.pop(0)()
                if j1q:
                    j1q.pop(0)()
            lb_o_ps, lb_qc, _lb_p = pending
            lb_rc = rc_pool.tile([128, 2, 4, 1], F32, tag="rc", name="lb_rc")
            lb_O = osb_pool.tile([128, 4, 2, 64], BF, tag="osb", name="lb_O")
            for hh in range(2):
                nc.vector.reciprocal(lb_rc[:, hh, :, :],
                                     lb_o_ps[hh][:, :, 64:65])
            # tail: last block's normalization; scales prefetched onto the
            # DVE while the PE chews leftover deferred projections, then a
            # hand-interleaved transpose/copy/projection pipeline.
            if deferred:
                deferred.pop(0)()
            # qi0's scales first so the first transpose (and the whole
            # per-qi tail pipeline behind it) starts half an op earlier
            for hh in range(2):
                nc.vector.tensor_scalar_mul(
                    lb_O[:, 0, hh, :], lb_o_ps[hh][:, 0, 0:64],
                    lb_rc[:, hh, 0, :])
            for hh in range(2):
                nc.vector.tensor_mul(
                    lb_O[:, 1:4, hh, :],
                    lb_o_ps[hh][:, 1:4, 0:64],
                    lb_rc[:, hh, 1:4, :].broadcast_to([128, 3, 64]))
            while deferred:
                deferred.pop(0)()
            tr_t = ps_qkv.tile([128, 4, 128], BF, tag="qkv", name="tr_ps_t")

            def tail_tr(qi):
                # the oT bias-add rides the otherwise-idle Pool engine so
                # the DVE is free for the projection copies
                nc.tensor.matmul(tr_t[:, qi, :], lb_O[:, qi, :, :],
                                 ident_sb[:, :], is_transpose=True,
                                 skip_group_check=True)
                nc.gpsimd.tensor_scalar_add(oT_sb[1][:, lb_qc, qi, :],
                                            tr_t[:, qi, :],
                                            bvp_sb[:, 1:2])

            tail_tr(0)
            tail_tr(1)
            for qi in range(4):
                qt = lb_qc * 4 + qi
                if qi + 2 < 4:
                    tail_tr(qi + 2)
                make_proj(qt, 1, copy=("act", "dve"), tail_pool=True)()

    nc.finalize()
    return nc


def make_core_inputs(x, W_qkv, b_qkv, W_proj, nt=NT):
    """Host-side shard prep: returns in_maps list for the 8 cores."""
    in_maps = []
    for core in range(NCORES):
        b, g = divmod(core, NCORES // B)
        lo, hi = g * DQ, (g + 1) * DQ
        xTb = np.ascontiguousarray(x[b].T).astype(BF16)
        wq_c = np.ascontiguousarray(W_qkv[:, lo:hi]).astype(BF16)
        wk_c = np.ascontiguousarray(W_qkv[:, C + lo:C + hi]).astype(BF16)
        wv_c = np.ascontiguousarray(W_qkv[:, 2 * C + lo:2 * C + hi]).astype(BF16)
        bvp_c = np.stack([
            b_qkv[2 * C + lo:2 * C + lo + 128],
            b_qkv[2 * C + lo + 128:2 * C + hi],
        ], axis=1).astype(np.float32)
        wp_c = np.ascontiguousarray(W_proj[lo:hi, :]).astype(BF16)
        bqk_c = np.stack([
            b_qkv[lo:lo + 128], b_qkv[lo + 128:hi],
            b_qkv[C + lo:C + lo + 128], b_qkv[C + lo + 128:C + hi],
        ], axis=1).astype(np.float32)
        in_maps.append({
            "xT": xTb[:, :nt].copy(), "wq": wq_c, "wk": wk_c, "wv": wv_c,
            "wp": wp_c, "bqk": bqk_c, "bvp": bvp_c,
        })
    return in_maps


_prog_cache = {}


def _get_program(nt=NT):
    if nt not in _prog_cache:
        _prog_cache[nt] = build_program(nt)
    return _prog_cache[nt]


def kernel(x, W_qkv, b_qkv, W_proj, b_proj, _run_kwargs=None):
    x = np.asarray(x, dtype=np.float32)
    W_qkv = np.asarray(W_qkv, dtype=np.float32)
    b_qkv = np.asarray(b_qkv, dtype=np.float32)
    W_proj = np.asarray(W_proj, dtype=np.float32)
    b_proj = np.asarray(b_proj, dtype=np.float32)

    nc = _get_program()
    in_maps = make_core_inputs(x, W_qkv, b_qkv, W_proj)
    for attempt in range(3):
        res = run_bass_kernel_spmd(nc, in_maps, core_ids=list(range(NCORES)),
                                   **(_run_kwargs or {}))
        out = np.zeros((B, NT, C), dtype=np.float32)
        for core in range(NCORES):
            b = core // (NCORES // B)
            part = np.asarray(res.results[core]["out_p"], dtype=np.float32)
            out[b] += part[0]
            # only the last query chunk is stored as two separate halves
            out[b, (NT // 4) * 3:] += part[1][(NT // 4) * 3:]
        if np.isfinite(out).all():
            break
        # transient device flake (observed rarely under axon): retry
    out += b_proj[None, None, :]
    if _run_kwargs:
        kernel.last_results = res
    return out

